# revision 3
# baseline (speedup 1.0000x reference)
"""Trainium2 Bass kernel for nn_LowFreqCrossAttn (dense cross-attention).

Data-parallel over batch: 16 batches -> 8 NeuronCores, 2 batches/core.
Weights / attention-bias tables replicated.

Per-core dataflow:
  A) untiled phase A: q = (s*Wq)@ll, k = Wk@ha into head-PAIR tiles
     [128, N] (head 2p at rows 0-63, 2p+1 at 64-127; 48 dims + 16 zero-pad
     rows); evac on DVE (ACT stays free for phase-B exp); vT = ha^T@WvT+vb
     (ones-row matmul) evacuated into pair-block layout [112, 4x128].
  B) attend with K=64 matmuls reading per-head partition halves directly
     (no head duplication): per (pair,batch,mi,hi) 2x2 (keys-chunk x nch)
     matmuls at 4 distinct (row_grp, col_grp) tile positions, interleaved
     across the two heads for PE tile concurrency. exp on ACT; exp(bias)
     multiply on DVE.
  C) per-pair tail: evac -> ounT; s rows -> reciprocal -> broadcast via
     DRAM bounce (sync queue) -> normalize on GpSimd (DVE for the last
     two units so GpSimd's end-of-program dge-drain overlaps compute).
  D) proj(b=0) interleaved into the last attend unit (PE stays warm);
     proj(b=1) right after the final tail chain.
"""

import numpy as np

B = 16
C = 384
RES = 28
N = 784
NH = 8
HD = 48
NP = 392            # n-chunk (half of N; fits one PSUM bank in f32)
NCORES = 8
BPC = 2             # batches per core
NPAIR = 4
SCALE = HD ** -0.5
MT = 112            # m-tile (7 x 112 = 784, uniform)
NMT = 7

TRACE = False
TRACE_DIR = None
LAST_RESULTS = {}

_CACHE = {}


def _build_nc():
    import concourse.bacc as bacc
    import concourse.mybir as mybir
    import concourse.tile as tile

    f16 = mybir.dt.float16
    f32 = mybir.dt.float32
    AF = mybir.ActivationFunctionType
    MUL = mybir.AluOpType.mult
    ADD = mybir.AluOpType.add

    nc = bacc.Bacc("TRN2", target_bir_lowering=False, debug=False)

    ll_d = nc.declare_dram_parameter("ll", [BPC, 128, 3 * N], f16, isOutput=False)
    ha_d = nc.declare_dram_parameter("ha", [BPC, 128, 3 * N], f16, isOutput=False)
    qwT_d = nc.declare_dram_parameter("qwT", [128, 3 * 512], f16, isOutput=False)
    kwT_d = nc.declare_dram_parameter("kwT", [128, 3 * 512], f16, isOutput=False)
    vwT_d = nc.declare_dram_parameter("vwT", [128, 3 * C], f16, isOutput=False)
    pwT_d = nc.declare_dram_parameter("pwT", [128, 4 * C], f16, isOutput=False)
    qb_d = nc.declare_dram_parameter("qb", [128, 4], f32, isOutput=False)
    kb_d = nc.declare_dram_parameter("kb", [128, 4], f32, isOutput=False)
    vb_d = nc.declare_dram_parameter("vb", [1, C], f16, isOutput=False)
    pb_d = nc.declare_dram_parameter("pb", [128, 3], f32, isOutput=False)
    # expb host layout: [pair, key-row within m-tile (112), mi (7), cols]
    expb_d = nc.declare_dram_parameter("expb", [NPAIR, MT, NMT * 1568], f16,
                                       isOutput=False)
    out_d = nc.declare_dram_parameter("out", [BPC, C, N], f32, isOutput=True)

    with tile.TileContext(nc) as tc:
        with (
            tc.tile_pool(name="const", bufs=1) as cp,
            tc.tile_pool(name="persist", bufs=1) as pp,
            tc.tile_pool(name="ebp", bufs=3) as ebp,
            tc.tile_pool(name="dram", bufs=1, space="DRAM") as dp,
        ):
            qwT_sb = cp.tile([128, 3, 512], f16, tag="qwT", name="qwT")
            kwT_sb = cp.tile([128, 3, 512], f16, tag="kwT", name="kwT")
            vwT_sb = cp.tile([128, 3, C], f16, tag="vwT", name="vwT")
            pwT_sb = cp.tile([128, 4, C], f16, tag="pwT", name="pwT")
            qb_sb = cp.tile([128, 4], f32, tag="qb", name="qb")
            kb_sb = cp.tile([128, 4], f32, tag="kb", name="kb")
            vb_sb = cp.tile([1, C], f16, tag="vb", name="vb")
            pb_sb = cp.tile([128, 3], f32, tag="pb", name="pb")
            # weights / small consts spread across queues
            nc.sync.dma_start(qwT_sb[:], qwT_d[:])
            nc.gpsimd.dma_start(kwT_sb[:], kwT_d[:])
            nc.gpsimd.dma_start(vwT_sb[:], vwT_d[:])
            nc.scalar.dma_start(pwT_sb[:], pwT_d[:])
            nc.sync.dma_start(qb_sb[:], qb_d[:])
            nc.sync.dma_start(kb_sb[:], kb_d[:])
            nc.gpsimd.dma_start(vb_sb[:], vb_d[:])
            nc.gpsimd.dma_start(pb_sb[:], pb_d[:])
            ones128 = cp.tile([1, 128], f16, tag="ones128", name="ones128")
            nc.gpsimd.memset(ones128[:], 1.0)
            ones2 = cp.tile([128, 128], f16, tag="ones2", name="ones2")
            nc.gpsimd.memset(ones2[:], 0.0)
            nc.gpsimd.memset(ones2[0:1, 0:64], 1.0)
            nc.gpsimd.memset(ones2[32:33, 64:128], 1.0)

            # q/k head-pair tiles: head 2p at rows 0:64, head 2p+1 at 64:128
            q_sb = [[pp.tile([128, N], f16, tag=f"q{b}_{p}", name=f"q{b}_{p}")
                     for p in range(NPAIR)] for b in range(BPC)]
            k_sb = [[pp.tile([128, N], f16, tag=f"k{b}_{p}", name=f"k{b}_{p}")
                     for p in range(NPAIR)] for b in range(BPC)]
            vT_sb = [[pp.tile([128, 512], f16, tag=f"vT{b}_{m}", name=f"vT{b}_{m}")
                      for m in range(NMT)] for b in range(BPC)]
            ounT = [[pp.tile([128, N], f16, tag=f"ounT{b}_{p}", name=f"ounT{b}_{p}")
                     for p in range(NPAIR)] for b in range(BPC)]
            onorm = [[pp.tile([128, N], f16, tag=f"onorm{b}_{p}", name=f"onorm{b}_{p}")
                      for p in range(NPAIR)] for b in range(BPC)]

            rd_dram = [[dp.tile([2, N], f16, tag=f"rd{b}_{p}", name=f"rd{b}_{p}")
                        for p in range(NPAIR)] for b in range(BPC)]

            eb_tiles = {}

            def load_eb(p):
                ebt = ebp.tile([128, NMT, 1568], f16, tag="eb", name="ebt")
                nc.scalar.dma_start(
                    ebt[0:MT, :, :].rearrange("p m c -> p (m c)"), expb_d[p])
                eb_tiles[p] = ebt

            # prefetch bias tiles for pairs 0/1 while phase A computes
            load_eb(0)
            load_eb(1)

            # one-time init: vT ones cols (block col 0 -> s lands at av rows
            # 0/64, partition-aligned for the tail chain)
            for b in range(BPC):
                for m in range(NMT):
                    vv = vT_sb[b][m].rearrange("m (p i c) -> m p i c", i=2, c=64)
                    nc.gpsimd.memset(vv[0:MT, :, :, 0:1], 1.0)
                    # pad cols zeroed so av pad rows stay finite
                    nc.gpsimd.memset(vv[0:MT, :, :, 49:64], 0.0)

            # ---- phase A (untiled): projections for both batches ----
            with (
                tc.tile_pool(name="actA", bufs=1) as apool,
                tc.tile_pool(name="psA", bufs=3, space="PSUM") as psA,
                tc.tile_pool(name="psV", bufs=2, space="PSUM") as psV,
            ):
                ll_sb = [apool.tile([128, 3, N], f16, tag=f"ll{b}", name=f"ll{b}")
                         for b in range(BPC)]
                ha_sb = [apool.tile([128, 3, N], f16, tag=f"ha{b}", name=f"ha{b}")
                         for b in range(BPC)]
                nc.sync.dma_start(
                    ll_sb[0][:].rearrange("p t n -> p (t n)"), ll_d[0])
                nc.gpsimd.dma_start(
                    ha_sb[0][:].rearrange("p t n -> p (t n)"), ha_d[0])
                nc.sync.dma_start(
                    ll_sb[1][:].rearrange("p t n -> p (t n)"), ll_d[1])
                nc.gpsimd.dma_start(
                    ha_sb[1][:].rearrange("p t n -> p (t n)"), ha_d[1])
                for b in range(BPC):
                    for (wt, bt, src_, dst) in (
                        (qwT_sb, qb_sb, ll_sb[b], q_sb[b]),
                        (kwT_sb, kb_sb, ha_sb[b], k_sb[b]),
                    ):
                        for p in range(4):
                            ps = psA.tile([128, 1024], f32, tag="qk", name="psqk")
                            for nch in range(2):
                                for t in range(3):
                                    nc.tensor.matmul(
                                        ps[:, 512 * nch:512 * nch + NP],
                                        wt[:, t, 128 * p:128 * (p + 1)],
                                        src_[:, t, NP * nch:NP * (nch + 1)],
                                        start=(t == 0),
                                        stop=(t == 2),
                                    )
                            nc.vector.tensor_scalar(
                                dst[p].rearrange("p (c n) -> p c n", c=2),
                                ps.rearrange("p (c n) -> p c n", n=512)[:, :, 0:NP],
                                bt[:, p:p + 1], None, ADD,
                            )
                    # vT projection -> pair-block layout
                    for mi in range(NMT):
                        off = MT * mi
                        ps = psV.tile([128, 384], f32, tag="vt", name="psvt")
                        nc.tensor.matmul(ps[0:MT, :], ones128[:, 0:MT], vb_sb[:],
                                         start=True, stop=False)
                        for t in range(3):
                            nc.tensor.matmul(
                                ps[0:MT, :],
                                ha_sb[b][:, t, off:off + MT],
                                vwT_sb[:, t, :],
                                start=False,
                                stop=(t == 2),
                            )
                        nc.vector.tensor_copy(
                            vT_sb[b][mi].rearrange("m (p i c) -> m p i c", i=2, c=64)
                            [0:MT, :, :, 1:49],
                            ps.rearrange("m (p i c) -> m p i c", i=2, c=48)[0:MT],
                        )

            # ---- phase B: attend, unit stream ----
            with (
                tc.tile_pool(name="etp", bufs=5) as etp,
                tc.tile_pool(name="tlp", bufs=2) as tlp,
                tc.tile_pool(name="psqk", bufs=3, space="PSUM") as psqk,
                tc.tile_pool(name="psav", bufs=1, space="PSUM") as psav,
            ):
                def emit_qk_exp_mult(p, b, mi, eT):
                    off = MT * mi
                    eb = eb_tiles[p]
                    kk = k_sb[b][p]
                    qq = q_sb[b][p]
                    qk = [psqk.tile([128, 1024], f32, tag="qk", name="qkt")
                          for _ in range(2)]
                    # 8 K=64 matmuls over 4 distinct (row_grp, col_grp) tile
                    # positions; consecutive 4 cover all positions so the PE
                    # can run them concurrently.
                    for nch in range(2):
                        for (hi, kc) in ((0, 0), (1, 1), (0, 1), (1, 0)):
                            r0 = 64 * hi
                            if kc == 0:
                                nc.tensor.matmul(
                                    qk[hi][0:64, 512 * nch:512 * nch + NP],
                                    kk[r0:r0 + 64, off:off + 64],
                                    qq[r0:r0 + 64, NP * nch:NP * (nch + 1)],
                                    start=True, stop=True,
                                )
                            else:
                                nc.tensor.matmul(
                                    qk[hi][64:112, 512 * nch:512 * nch + NP],
                                    kk[r0:r0 + 64, off + 64:off + MT],
                                    qq[r0:r0 + 64, NP * nch:NP * (nch + 1)],
                                    start=True, stop=True,
                                )
                    for hi in range(2):
                        nc.scalar.activation(
                            eT.rearrange("m (h c n) -> m h c n", h=2, n=NP)
                            [0:MT, hi],
                            qk[hi].rearrange("m (c n) -> m c n", n=512)
                            [0:MT, :, 0:NP],
                            AF.Exp)
                    nc.vector.tensor_tensor(
                        eT[0:MT, :], eT[0:MT, :], eb[0:MT, mi, :], MUL)

                def emit_av(p, b, mi, eT, av):
                    for hi in range(2):
                        col = 64 * hi
                        for nch in range(2):
                            nc.tensor.matmul(
                                av[col:col + 64, 512 * nch:512 * nch + NP],
                                vT_sb[b][mi][0:MT,
                                             128 * p + col:128 * p + col + 64],
                                eT.rearrange("m (h c n) -> m h c n",
                                             h=2, n=NP)[0:MT, hi, nch],
                                start=(mi == 0), stop=(mi == NMT - 1),
                            )

                def tail(p, b, av, pos):
                    ave = av.rearrange("m (c n) -> m c n", n=512)
                    ou = ounT[b][p].rearrange("m (c n) -> m c n", n=NP)
                    nc.vector.tensor_copy(ou, ave[:, :, 0:NP])
                    # s rows (ounT rows 0/64, aligned) -> recip -> r16.
                    s32 = tlp.tile([128, N], f32, tag="s32", name="s32t")
                    r32 = tlp.tile([128, N], f32, tag="r32", name="r32t")
                    r16 = tlp.tile([128, N], f16, tag="r16", name="r16t")
                    bc = tlp.tile([128, N], f16, tag="bc", name="bct")
                    last = pos == 2 * NPAIR - 1
                    nr = 65 if last else 33
                    if last:
                        # junk rows finite so the PE broadcast can't make NaN
                        nc.vector.memset(s32[0:96, :], 1.0)
                    nc.vector.tensor_copy(s32[0:1, :], ounT[b][p][0:1, :])
                    nc.vector.tensor_copy(s32[32:33, :], ounT[b][p][64:65, :])
                    nc.vector.reciprocal_approx_fast(r32[0:nr, :], s32[0:nr, :])
                    nc.vector.tensor_copy(r16[0:nr, :], r32[0:nr, :])
                    if last:
                        # broadcast r on the PE (K=65 rounds to the 128-row
                        # mode; ones2's zero rows kill the junk)
                        bcp = psav.tile([128, 1024], f32, tag="av", name="bcp")
                        for nch in range(2):
                            nc.tensor.matmul(
                                bcp[0:64, 512 * nch:512 * nch + NP],
                                ones2[0:65, 0:64],
                                r16[0:65, NP * nch:NP * (nch + 1)],
                                start=True, stop=True)
                            nc.tensor.matmul(
                                bcp[64:128, 512 * nch:512 * nch + NP],
                                ones2[0:65, 64:128],
                                r16[0:65, NP * nch:NP * (nch + 1)],
                                start=True, stop=True)
                        bv = bcp.rearrange("m (c n) -> m c n", n=512)
                        ov = onorm[b][p].rearrange("m (c n) -> m c n", n=NP)
                        uv = ounT[b][p].rearrange("m (c n) -> m c n", n=NP)
                        nc.vector.tensor_tensor(
                            ov[0:64], uv[0:64], bv[0:64, :, 0:NP], MUL)
                        nc.vector.tensor_tensor(
                            ov[64:128], uv[64:128], bv[64:128, :, 0:NP], MUL)
                        return
                    # partition-broadcast r16 rows 0/32 via DRAM bounce
                    nc.sync.dma_start(
                        rd_dram[b][p][:],
                        r16.rearrange("(a z) n -> a z n", z=32)[0:2, 0:1, :])
                    rd_ap = rd_dram[b][p].tensor.ap()
                    nc.sync.dma_start(
                        bc[0:64, :], rd_ap[0:1, :].to_broadcast((64, N)))
                    nc.sync.dma_start(
                        bc[64:128, :], rd_ap[1:2, :].to_broadcast((64, N)))
                    # normalize (rows 0/64 hold s*r garbage; killed by zero
                    # rows in pwT); last-but-one on DVE so GpSimd's program
                    # ends early and its dge-drain overlaps compute
                    eng = nc.vector if pos >= 2 * NPAIR - 2 else nc.gpsimd
                    eng.tensor_tensor(
                        onorm[b][p][0:64, :], ounT[b][p][0:64, :],
                        bc[0:64, :], MUL)
                    eng.tensor_tensor(
                        onorm[b][p][64:128, :], ounT[b][p][64:128, :],
                        bc[64:128, :], MUL)

                def proj_o(b, o, ypool):
                    ps = psqk.tile([128, 1024], f32, tag="qk", name="psy")
                    for nch in range(2):
                        for p in range(NPAIR):
                            nc.tensor.matmul(
                                ps[:, 512 * nch:512 * nch + NP],
                                pwT_sb[:, p, 128 * o:128 * (o + 1)],
                                onorm[b][p][:, NP * nch:NP * (nch + 1)],
                                start=(p == 0), stop=(p == 3),
                            )
                    y_sb = ypool.tile([128, N], f32, tag="y", name="ysb")
                    nc.vector.tensor_scalar(
                        y_sb.rearrange("m (c n) -> m c n", n=NP),
                        ps.rearrange("m (c n) -> m c n", n=512)[:, :, 0:NP],
                        pb_sb[:, o:o + 1], None, ADD)
                    nc.sync.dma_start(
                        out_d[b, 128 * o:128 * (o + 1), :], y_sb[:])

                with tc.tile_pool(name="yp", bufs=3) as ypool:
                    sched = []
                    for p in range(NPAIR):
                        sched.append((p, 0))
                        if p >= 1:
                            sched.append((p - 1, 1))
                    sched.append((NPAIR - 1, 1))
                    # flat unit stream with AV lagging one m-tile
                    pending = []  # (p, b, mi, eT, av) awaiting AV emit
                    unit_pos = {u: i for i, u in enumerate(sched)}

                    def flush_one():
                        a = pending.pop(0)
                        emit_av(*a)
                        if a[2] == NMT - 1:
                            tail(a[0], a[1], a[4], unit_pos[(a[0], a[1])])

                    for (p, b) in sched:
                        if b == 0 and p + 2 < NPAIR and (p + 2) not in eb_tiles:
                            load_eb(p + 2)
                        av = psav.tile([128, 1024], f32, tag="av", name="avt")
                        lastu = (p, b) == sched[-1]
                        for mi in range(NMT):
                            eT = etp.tile([128, 1568], f16, tag="eT", name="eTt")
                            emit_qk_exp_mult(p, b, mi, eT)
                            # proj(0) interleaved into the last unit's stream:
                            # keeps the PE warm through the endgame
                            if lastu and mi in (1, 3, 5):
                                proj_o(0, (mi - 1) // 2, ypool)
                            maxp = 1 if lastu else 2
                            while len(pending) > maxp:
                                flush_one()
                            pending.append((p, b, mi, eT, av))
                    while pending:
                        flush_one()
                    for o in range(3):
                        proj_o(1, o, ypool)

    nc.finalize()
    return nc


def _prep_consts(q_w, q_b, kv_w, kv_b, proj_w, proj_b, attn_biases, bias_idxs):
    f16 = np.float16
    qw = (q_w * SCALE).astype(np.float32)
    qb = (q_b * SCALE).astype(np.float32)
    kw = kv_w[:C].astype(np.float32)
    kb = kv_b[:C].astype(np.float32)
    vw = kv_w[C:]
    vb = kv_b[C:]

    def pad64(w2, b1):  # [384(o), 384(c)] -> [512, 384] / [512]
        wp = np.zeros((512, C), np.float32)
        bp = np.zeros((512,), np.float32)
        for h in range(NH):
            wp[64 * h:64 * h + HD] = w2[HD * h:HD * (h + 1)]
            bp[64 * h:64 * h + HD] = b1[HD * h:HD * (h + 1)]
        return wp, bp

    qwp, qbp = pad64(qw, qb)
    kwp, kbp = pad64(kw, kb)
    # DRAM layout [128, 3, 512]: partition r, chunk t = w row 128t + r
    qwT = np.ascontiguousarray(
        qwp.T.reshape(3, 128, 512).transpose(1, 0, 2).reshape(128, 3 * 512)
    ).astype(f16)
    kwT = np.ascontiguousarray(
        kwp.T.reshape(3, 128, 512).transpose(1, 0, 2).reshape(128, 3 * 512)
    ).astype(f16)
    vwT = np.ascontiguousarray(
        vw.T.reshape(3, 128, C).transpose(1, 0, 2).reshape(128, 3 * C)
    ).astype(f16)

    # proj weights: pair tile p rows 0:48 = head 2p dims, 64:112 = head 2p+1
    pwT = np.zeros((4, 128, C), np.float32)
    for p in range(4):
        pwT[p, 1:1 + HD] = proj_w[:, 96 * p:96 * p + HD].T
        pwT[p, 65:65 + HD] = proj_w[:, 96 * p + HD:96 * p + 96].T
    pwT = np.ascontiguousarray(
        pwT.transpose(1, 0, 2).reshape(128, 4 * C)).astype(f16)

    qb_h = np.ascontiguousarray(qbp.reshape(4, 128).T).astype(np.float32)
    kb_h = np.ascontiguousarray(kbp.reshape(4, 128).T).astype(np.float32)
    pb_h = np.ascontiguousarray(proj_b.reshape(3, 128).T).astype(np.float32)
    vb_h = vb.reshape(1, C).astype(f16)

    eb = np.exp(attn_biases[:, bias_idxs]).astype(np.float32)  # [8, N, N]
    # pair-interleave: [p, m, (2i+c)*392+j] = eb[2p+i, m, 392c+j]
    e4 = eb.reshape(NPAIR, 2, N, 2, NP)           # [p, i, m, c, j]
    expb = e4.transpose(0, 2, 1, 3, 4).reshape(NPAIR, N, 1568)
    # DRAM layout [pair, 112, 7*1568]: partition r, mi chunk = key 112*mi + r
    expb = np.ascontiguousarray(
        expb.reshape(NPAIR, NMT, MT, 1568).transpose(0, 2, 1, 3)
        .reshape(NPAIR, MT, NMT * 1568)).astype(f16)

    return dict(qwT=qwT, kwT=kwT, vwT=vwT, pwT=pwT, qb=qb_h, kb=kb_h,
                vb=vb_h, pb=pb_h, expb=expb)


def kernel(ll, high_attn, q_w, q_b, kv_w, kv_b, proj_w, proj_b,
           attn_biases, bias_idxs):
    from concourse.bass_utils import run_bass_kernel_spmd

    global LAST_RESULTS
    ll = np.asarray(ll)
    high_attn = np.asarray(high_attn)

    if "nc" not in _CACHE:
        _CACHE["nc"] = _build_nc()
    nc = _CACHE["nc"]

    consts = _prep_consts(
        np.asarray(q_w), np.asarray(q_b), np.asarray(kv_w), np.asarray(kv_b),
        np.asarray(proj_w), np.asarray(proj_b), np.asarray(attn_biases),
        np.asarray(bias_idxs),
    )

    # DRAM layout [B, 128, 3*784]: partition r, chunk t = channel 128t + r
    ll16 = np.ascontiguousarray(
        ll.reshape(B, 3, 128, N).transpose(0, 2, 1, 3)
        .reshape(B, 128, 3 * N)).astype(np.float16)
    ha16 = np.ascontiguousarray(
        high_attn.reshape(B, 3, 128, N).transpose(0, 2, 1, 3)
        .reshape(B, 128, 3 * N)).astype(np.float16)

    in_maps = []
    for i in range(NCORES):
        m = {"ll": ll16[BPC * i:BPC * (i + 1)], "ha": ha16[BPC * i:BPC * (i + 1)]}
        m.update(consts)
        in_maps.append(m)

    res = run_bass_kernel_spmd(nc, in_maps, core_ids=list(range(NCORES)),
                               trace=TRACE, tmpdir=TRACE_DIR)
    LAST_RESULTS = {"exec_time_ns": res.exec_time_ns,
                    "scope_times": res.per_core_scope_times}

    out = np.empty((B, C, N), np.float32)
    for i in range(NCORES):
        out[BPC * i:BPC * (i + 1)] = res.results[i]["out"]
    return out.reshape(B, C, RES, RES)


# revision 14
# speedup vs baseline: 1.1384x; 1.1384x over previous
"""Trainium2 Bass kernel for nn_LowFreqCrossAttn (dense cross-attention).

Data-parallel over batch: 16 batches -> 8 NeuronCores, 2 batches/core.
Weights / attention-bias tables replicated.

Per-core dataflow:
  A) untiled phase A: q = (s*Wq)@ll, k = Wk@ha into head-PAIR tiles
     [128, N] (head 2p at rows 0-63, 2p+1 at 64-127; 48 dims + 16 zero-pad
     rows); evac on DVE (ACT stays free for phase-B exp); vT = ha^T@WvT+vb
     (ones-row matmul) evacuated into pair-block layout [112, 4x128].
  B) attend with K=64 matmuls reading per-head partition halves directly
     (no head duplication): per (pair,batch,mi,hi) 2x2 (keys-chunk x nch)
     matmuls at 4 distinct (row_grp, col_grp) tile positions, interleaved
     across the two heads for PE tile concurrency. exp on ACT; exp(bias)
     multiply on DVE.
  C) per-pair tail: evac -> ounT; s rows -> reciprocal -> broadcast via
     DRAM bounce (sync queue) -> normalize on GpSimd (DVE for the last
     two units so GpSimd's end-of-program dge-drain overlaps compute).
  D) proj(b=0) interleaved into the last attend unit (PE stays warm);
     proj(b=1) right after the final tail chain.
"""

import numpy as np

B = 16
C = 384
RES = 28
N = 784
NH = 8
HD = 48
NP = 392            # n-chunk (half of N; fits one PSUM bank in f32)
NCORES = 8
BPC = 2             # batches per core
NPAIR = 4
SCALE = HD ** -0.5
MT = 112            # m-tile (7 x 112 = 784, uniform)
NMT = 7

TRACE = False
TRACE_DIR = None
LAST_RESULTS = {}

_CACHE = {}


def _build_nc():
    import concourse.bacc as bacc
    import concourse.mybir as mybir
    import concourse.tile as tile

    f16 = mybir.dt.float16
    f32 = mybir.dt.float32
    AF = mybir.ActivationFunctionType
    MUL = mybir.AluOpType.mult
    ADD = mybir.AluOpType.add

    nc = bacc.Bacc("TRN2", target_bir_lowering=False, debug=False)

    ll_d = nc.declare_dram_parameter("ll", [BPC, C, N], f16, isOutput=False)
    ha_d = nc.declare_dram_parameter("ha", [BPC, C, N], f16, isOutput=False)
    qwT_d = nc.declare_dram_parameter("qwT", [3, 128, 512], f16, isOutput=False)
    kwT_d = nc.declare_dram_parameter("kwT", [3, 128, 512], f16, isOutput=False)
    vwT_d = nc.declare_dram_parameter("vwT", [128, 3 * C], f16, isOutput=False)
    pwT_d = nc.declare_dram_parameter("pwT", [128, 4 * C], f16, isOutput=False)
    qb_d = nc.declare_dram_parameter("qb", [128, 4], f32, isOutput=False)
    kb_d = nc.declare_dram_parameter("kb", [128, 4], f32, isOutput=False)
    vb_d = nc.declare_dram_parameter("vb", [1, C], f16, isOutput=False)
    pb_d = nc.declare_dram_parameter("pb", [128, 3], f32, isOutput=False)
    # expb host layout: [pair, key-row within m-tile (112), mi (7), cols]
    expb_d = nc.declare_dram_parameter("expb", [NPAIR, MT, NMT * 1568], f16,
                                       isOutput=False)
    out_d = nc.declare_dram_parameter("out", [BPC, C, N], f32, isOutput=True)

    with tile.TileContext(nc) as tc:
        with (
            tc.tile_pool(name="const", bufs=1) as cp,
            tc.tile_pool(name="persist", bufs=1) as pp,
            tc.tile_pool(name="ebp", bufs=3) as ebp,
            tc.tile_pool(name="dram", bufs=1, space="DRAM") as dp,
        ):
            qwT_sb = [cp.tile([128, 512], f16, tag=f"qwT{t}", name=f"qwT{t}")
                      for t in range(3)]
            kwT_sb = [cp.tile([128, 512], f16, tag=f"kwT{t}", name=f"kwT{t}")
                      for t in range(3)]
            vwT_sb = cp.tile([128, 3, C], f16, tag="vwT", name="vwT")
            pwT_sb = cp.tile([128, 4, C], f16, tag="pwT", name="pwT")
            qb_sb = cp.tile([128, 4], f32, tag="qb", name="qb")
            kb_sb = cp.tile([128, 4], f32, tag="kb", name="kb")
            vb_sb = cp.tile([1, C], f16, tag="vb", name="vb")
            pb_sb = cp.tile([128, 3], f32, tag="pb", name="pb")
            # weights / small consts spread across queues
            for t in range(3):
                nc.sync.dma_start(qwT_sb[t][:], qwT_d[t])
                nc.gpsimd.dma_start(kwT_sb[t][:], kwT_d[t])
            nc.gpsimd.dma_start(vwT_sb[:], vwT_d[:])
            nc.scalar.dma_start(pwT_sb[:], pwT_d[:])
            nc.sync.dma_start(qb_sb[:], qb_d[:])
            nc.sync.dma_start(kb_sb[:], kb_d[:])
            nc.gpsimd.dma_start(vb_sb[:], vb_d[:])
            nc.gpsimd.dma_start(pb_sb[:], pb_d[:])
            ones128 = cp.tile([1, 128], f16, tag="ones128", name="ones128")
            nc.gpsimd.memset(ones128[:], 1.0)
            ones2 = cp.tile([128, 128], f16, tag="ones2", name="ones2")
            nc.gpsimd.memset(ones2[:], 0.0)
            nc.gpsimd.memset(ones2[0:1, 0:64], 1.0)
            nc.gpsimd.memset(ones2[32:33, 64:128], 1.0)

            # q/k head-pair tiles: head 2p at rows 0:64, head 2p+1 at 64:128
            q_sb = [[pp.tile([128, N], f16, tag=f"q{b}_{p}", name=f"q{b}_{p}")
                     for p in range(NPAIR)] for b in range(BPC)]
            k_sb = [[pp.tile([128, N], f16, tag=f"k{b}_{p}", name=f"k{b}_{p}")
                     for p in range(NPAIR)] for b in range(BPC)]
            vT_sb = [[pp.tile([128, 512], f16, tag=f"vT{b}_{m}", name=f"vT{b}_{m}")
                      for m in range(NMT)] for b in range(BPC)]
            ounT = [[pp.tile([128, N], f16, tag=f"ounT{b}_{p}", name=f"ounT{b}_{p}")
                     for p in range(NPAIR)] for b in range(BPC)]
            onorm = [[pp.tile([128, N], f16, tag=f"onorm{b}_{p}", name=f"onorm{b}_{p}")
                      for p in range(NPAIR)] for b in range(BPC)]

            rd_dram = [[dp.tile([2, N], f32, tag=f"rd{b}_{p}", name=f"rd{b}_{p}")
                        for p in range(NPAIR)] for b in range(BPC)]

            eb_tiles = {}

            def load_eb(p):
                ebt = ebp.tile([128, NMT, 1568], f16, tag="eb", name="ebt")
                nc.sync.dma_start(
                    ebt[0:MT, :, :].rearrange("p m c -> p (m c)"), expb_d[p])
                eb_tiles[p] = ebt

            # one-time init: vT ones cols (block col 0 -> s lands at av rows
            # 0/64, partition-aligned for the tail chain)
            for b in range(BPC):
                for m in range(NMT):
                    vv = vT_sb[b][m].rearrange("m (p i c) -> m p i c", i=2, c=64)
                    nc.gpsimd.memset(vv[0:MT, :, :, 0:1], 1.0)
                    # pad cols zeroed so av pad rows stay finite
                    nc.gpsimd.memset(vv[0:MT, :, :, 49:64], 0.0)

            # ---- phase A (untiled): projections for both batches ----
            with (
                tc.tile_pool(name="actA", bufs=1) as apool,
                tc.tile_pool(name="psA", bufs=3, space="PSUM") as psA,
                tc.tile_pool(name="psV", bufs=2, space="PSUM") as psV,
            ):
                ll_sb = [[apool.tile([128, N], f16, tag=f"ll{b}_{t}", name=f"ll{b}_{t}")
                          for t in range(3)] for b in range(BPC)]
                ha_sb = [[apool.tile([128, N], f16, tag=f"ha{b}_{t}", name=f"ha{b}_{t}")
                          for t in range(3)] for b in range(BPC)]
                for t in range(3):
                    nc.sync.dma_start(ll_sb[0][t][:], ll_d[0, 128 * t:128 * (t + 1), :])
                    nc.gpsimd.dma_start(ha_sb[0][t][:], ha_d[0, 128 * t:128 * (t + 1), :])
                for t in range(3):
                    nc.sync.dma_start(ll_sb[1][t][:], ll_d[1, 128 * t:128 * (t + 1), :])
                    nc.gpsimd.dma_start(ha_sb[1][t][:], ha_d[1, 128 * t:128 * (t + 1), :])
                # bias-tile prefetch rides the DMA engines behind the inputs
                load_eb(0)
                for b in range(BPC):
                    for (wt, bt, src_, dst) in (
                        (qwT_sb, qb_sb, ll_sb[b], q_sb[b]),
                        (kwT_sb, kb_sb, ha_sb[b], k_sb[b]),
                    ):
                        for p in range(4):
                            ps = psA.tile([128, 1024], f32, tag="qk", name="psqk")
                            for nch in range(2):
                                for t in range(3):
                                    nc.tensor.matmul(
                                        ps[:, 512 * nch:512 * nch + NP],
                                        wt[t][:, 128 * p:128 * (p + 1)],
                                        src_[t][:, NP * nch:NP * (nch + 1)],
                                        start=(t == 0),
                                        stop=(t == 2),
                                    )
                            nc.vector.tensor_scalar(
                                dst[p].rearrange("p (c n) -> p c n", c=2),
                                ps.rearrange("p (c n) -> p c n", n=512)[:, :, 0:NP],
                                bt[:, p:p + 1], None, ADD,
                            )
                    # vT projection -> pair-block layout
                    for mi in range(NMT):
                        off = MT * mi
                        ps = psV.tile([128, 384], f32, tag="vt", name="psvt")
                        nc.tensor.matmul(ps[0:MT, :], ones128[:, 0:MT], vb_sb[:],
                                         start=True, stop=False)
                        for t in range(3):
                            nc.tensor.matmul(
                                ps[0:MT, :],
                                ha_sb[b][t][:, off:off + MT],
                                vwT_sb[:, t, :],
                                start=False,
                                stop=(t == 2),
                            )
                        nc.vector.tensor_copy(
                            vT_sb[b][mi].rearrange("m (p i c) -> m p i c", i=2, c=64)
                            [0:MT, :, :, 1:49],
                            ps.rearrange("m (p i c) -> m p i c", i=2, c=48)[0:MT],
                        )
                    if b == 0:
                        load_eb(1)

            # ---- phase B: attend, unit stream ----
            with (
                tc.tile_pool(name="etp", bufs=5) as etp,
                tc.tile_pool(name="tlp", bufs=2) as tlp,
                tc.tile_pool(name="psqk", bufs=3, space="PSUM") as psqk,
                tc.tile_pool(name="psav", bufs=1, space="PSUM") as psav,
            ):
                def emit_qk_exp_mult(p, b, mi, eT):
                    off = MT * mi
                    eb = eb_tiles[p]
                    kk = k_sb[b][p]
                    qq = q_sb[b][p]
                    # K=64 matmuls read the head's partition half directly;
                    # the two key-chunk col groups run concurrently, and the
                    # h1 group's ldweights (row_grp 64) pulls ahead of h0's
                    # in-flight matmuls (row_grp 0).
                    for hi in range(2):
                        qk = psqk.tile([128, 1024], f32, tag="qk", name="qkt")
                        r0 = 64 * hi
                        for nch in range(2):
                            nc.tensor.matmul(
                                qk[0:64, 512 * nch:512 * nch + NP],
                                kk[r0:r0 + 64, off:off + 64],
                                qq[r0:r0 + 64, NP * nch:NP * (nch + 1)],
                                start=True, stop=True,
                            )
                            nc.tensor.matmul(
                                qk[64:112, 512 * nch:512 * nch + NP],
                                kk[r0:r0 + 64, off + 64:off + MT],
                                qq[r0:r0 + 64, NP * nch:NP * (nch + 1)],
                                start=True, stop=True,
                            )
                        nc.scalar.activation(
                            eT.rearrange("m (h c n) -> m h c n", h=2, n=NP)
                            [0:MT, hi],
                            qk.rearrange("m (c n) -> m c n", n=512)
                            [0:MT, :, 0:NP],
                            AF.Exp)
                    nc.vector.tensor_tensor(
                        eT[0:MT, :], eT[0:MT, :], eb[0:MT, mi, :], MUL)

                def emit_av(p, b, mi, eT, av):
                    for hi in range(2):
                        col = 64 * hi
                        for nch in range(2):
                            nc.tensor.matmul(
                                av[col:col + 64, 512 * nch:512 * nch + NP],
                                vT_sb[b][mi][0:MT,
                                             128 * p + col:128 * p + col + 64],
                                eT.rearrange("m (h c n) -> m h c n",
                                             h=2, n=NP)[0:MT, hi, nch],
                                start=(mi == 0), stop=(mi == NMT - 1),
                            )

                def tail(p, b, av, pos):
                    ave = av.rearrange("m (c n) -> m c n", n=512)
                    ou = ounT[b][p].rearrange("m (c n) -> m c n", n=NP)
                    nc.vector.tensor_copy(ou, ave[:, :, 0:NP])
                    last = pos == 2 * NPAIR - 1
                    if not last:
                        # s sums sit in f32 psum rows 0/64: recip straight off
                        # the psum (junk rows 1:63 computed but never read),
                        # bounce rows 0/64 through DRAM, broadcast as f32
                        r32 = tlp.tile([128, N], f32, tag="r32", name="r32t")
                        bc = tlp.tile([128, N], f32, tag="bc", name="bct")
                        nc.vector.reciprocal_approx_fast(
                            r32.rearrange("m (c n) -> m c n", n=NP)[0:65],
                            ave[0:65, :, 0:NP])
                        nc.sync.dma_start(
                            rd_dram[b][p][:],
                            r32.rearrange("(a z) n -> a z n", z=64)[0:2, 0:1, :])
                        rd_ap = rd_dram[b][p].tensor.ap()
                        nc.sync.dma_start(
                            bc[0:64, :], rd_ap[0:1, :].to_broadcast((64, N)))
                        nc.sync.dma_start(
                            bc[64:128, :], rd_ap[1:2, :].to_broadcast((64, N)))
                        # normalize (rows 0/64 hold s*r garbage; killed by
                        # zero rows in pwT); last-but-one on DVE so GpSimd's
                        # program ends early and its dge-drain overlaps
                        eng = nc.vector if pos >= 2 * NPAIR - 2 else nc.gpsimd
                        eng.tensor_tensor(
                            onorm[b][p][0:64, :], ounT[b][p][0:64, :],
                            bc[0:64, :], MUL)
                        eng.tensor_tensor(
                            onorm[b][p][64:128, :], ounT[b][p][64:128, :],
                            bc[64:128, :], MUL)
                        return
                    # final unit: broadcast r on the PE instead
                    s32 = tlp.tile([128, N], f32, tag="s32", name="s32t")
                    r32 = tlp.tile([128, N], f32, tag="r32", name="r32t")
                    r16 = tlp.tile([128, N], f16, tag="r16", name="r16t")
                    nr = 65
                    # junk rows finite so the PE broadcast can't make NaN
                    nc.vector.memset(s32[0:96, :], 1.0)
                    nc.vector.tensor_copy(s32[0:1, :], ounT[b][p][0:1, :])
                    nc.vector.tensor_copy(s32[32:33, :], ounT[b][p][64:65, :])
                    nc.vector.reciprocal_approx_fast(r32[0:nr, :], s32[0:nr, :])
                    nc.vector.tensor_copy(r16[0:nr, :], r32[0:nr, :])
                    # broadcast r on the PE (K=65 rounds to the 128-row
                    # mode; ones2's zero rows kill the junk)
                    bcp = psav.tile([128, 1024], f32, tag="av", name="bcp")
                    for nch in range(2):
                        nc.tensor.matmul(
                            bcp[0:64, 512 * nch:512 * nch + NP],
                            ones2[0:65, 0:64],
                            r16[0:65, NP * nch:NP * (nch + 1)],
                            start=True, stop=True)
                        nc.tensor.matmul(
                            bcp[64:128, 512 * nch:512 * nch + NP],
                            ones2[0:65, 64:128],
                            r16[0:65, NP * nch:NP * (nch + 1)],
                            start=True, stop=True)
                    bv = bcp.rearrange("m (c n) -> m c n", n=512)
                    ov = onorm[b][p].rearrange("m (c n) -> m c n", n=NP)
                    uv = ounT[b][p].rearrange("m (c n) -> m c n", n=NP)
                    nc.vector.tensor_tensor(
                        ov[0:64], uv[0:64], bv[0:64, :, 0:NP], MUL)
                    nc.vector.tensor_tensor(
                        ov[64:128], uv[64:128], bv[64:128, :, 0:NP], MUL)

                def proj_o(b, o, ypool):
                    ps = psqk.tile([128, 1024], f32, tag="qk", name="psy")
                    for nch in range(2):
                        for p in range(NPAIR):
                            nc.tensor.matmul(
                                ps[:, 512 * nch:512 * nch + NP],
                                pwT_sb[:, p, 128 * o:128 * (o + 1)],
                                onorm[b][p][:, NP * nch:NP * (nch + 1)],
                                start=(p == 0), stop=(p == 3),
                            )
                    y_sb = ypool.tile([128, N], f32, tag="y", name="ysb")
                    nc.vector.tensor_scalar(
                        y_sb.rearrange("m (c n) -> m c n", n=NP),
                        ps.rearrange("m (c n) -> m c n", n=512)[:, :, 0:NP],
                        pb_sb[:, o:o + 1], None, ADD)
                    nc.sync.dma_start(
                        out_d[b, 128 * o:128 * (o + 1), :], y_sb[:])

                with tc.tile_pool(name="yp", bufs=3) as ypool:
                    sched = []
                    for p in range(NPAIR):
                        sched.append((p, 0))
                        if p >= 1:
                            sched.append((p - 1, 1))
                    sched.append((NPAIR - 1, 1))
                    # flat unit stream with AV lagging one m-tile
                    pending = []  # (p, b, mi, eT, av) awaiting AV emit
                    unit_pos = {u: i for i, u in enumerate(sched)}

                    def flush_one():
                        a = pending.pop(0)
                        emit_av(*a)
                        if a[2] == NMT - 1:
                            tail(a[0], a[1], a[4], unit_pos[(a[0], a[1])])

                    for (p, b) in sched:
                        if b == 0 and p + 2 < NPAIR and (p + 2) not in eb_tiles:
                            load_eb(p + 2)
                        av = psav.tile([128, 1024], f32, tag="av", name="avt")
                        lastu = (p, b) == sched[-1]
                        for mi in range(NMT):
                            eT = etp.tile([128, 1568], f16, tag="eT", name="eTt")
                            emit_qk_exp_mult(p, b, mi, eT)
                            # proj(0) interleaved into the last unit's stream:
                            # keeps the PE warm through the endgame
                            if lastu and mi in (1, 3, 5):
                                proj_o(0, (mi - 1) // 2, ypool)
                            maxp = 1 if lastu else 2
                            while len(pending) > maxp:
                                flush_one()
                            pending.append((p, b, mi, eT, av))
                    while pending:
                        flush_one()
                    for o in range(3):
                        proj_o(1, o, ypool)

    nc.finalize()
    return nc


def _prep_consts(q_w, q_b, kv_w, kv_b, proj_w, proj_b, attn_biases, bias_idxs):
    f16 = np.float16
    qw = (q_w * SCALE).astype(np.float32)
    qb = (q_b * SCALE).astype(np.float32)
    kw = kv_w[:C].astype(np.float32)
    kb = kv_b[:C].astype(np.float32)
    vw = kv_w[C:]
    vb = kv_b[C:]

    def pad64(w2, b1):  # [384(o), 384(c)] -> [512, 384] / [512]
        wp = np.zeros((512, C), np.float32)
        bp = np.zeros((512,), np.float32)
        for h in range(NH):
            wp[64 * h:64 * h + HD] = w2[HD * h:HD * (h + 1)]
            bp[64 * h:64 * h + HD] = b1[HD * h:HD * (h + 1)]
        return wp, bp

    qwp, qbp = pad64(qw, qb)
    kwp, kbp = pad64(kw, kb)
    qwT = np.ascontiguousarray(qwp.T.reshape(3, 128, 512)).astype(f16)
    kwT = np.ascontiguousarray(kwp.T.reshape(3, 128, 512)).astype(f16)
    # vwT DRAM layout [128, 3, C]: partition r, chunk t = w row 128t + r
    vwT = np.ascontiguousarray(
        vw.T.reshape(3, 128, C).transpose(1, 0, 2).reshape(128, 3 * C)
    ).astype(f16)

    # proj weights: pair tile p rows 0:48 = head 2p dims, 64:112 = head 2p+1
    pwT = np.zeros((4, 128, C), np.float32)
    for p in range(4):
        pwT[p, 1:1 + HD] = proj_w[:, 96 * p:96 * p + HD].T
        pwT[p, 65:65 + HD] = proj_w[:, 96 * p + HD:96 * p + 96].T
    pwT = np.ascontiguousarray(
        pwT.transpose(1, 0, 2).reshape(128, 4 * C)).astype(f16)

    qb_h = np.ascontiguousarray(qbp.reshape(4, 128).T).astype(np.float32)
    kb_h = np.ascontiguousarray(kbp.reshape(4, 128).T).astype(np.float32)
    pb_h = np.ascontiguousarray(proj_b.reshape(3, 128).T).astype(np.float32)
    vb_h = vb.reshape(1, C).astype(f16)

    eb = np.exp(attn_biases[:, bias_idxs]).astype(np.float32)  # [8, N, N]
    # pair-interleave: [p, m, (2i+c)*392+j] = eb[2p+i, m, 392c+j]
    e4 = eb.reshape(NPAIR, 2, N, 2, NP)           # [p, i, m, c, j]
    expb = e4.transpose(0, 2, 1, 3, 4).reshape(NPAIR, N, 1568)
    # DRAM layout [pair, 112, 7*1568]: partition r, mi chunk = key 112*mi + r
    expb = np.ascontiguousarray(
        expb.reshape(NPAIR, NMT, MT, 1568).transpose(0, 2, 1, 3)
        .reshape(NPAIR, MT, NMT * 1568)).astype(f16)

    return dict(qwT=qwT, kwT=kwT, vwT=vwT, pwT=pwT, qb=qb_h, kb=kb_h,
                vb=vb_h, pb=pb_h, expb=expb)


def kernel(ll, high_attn, q_w, q_b, kv_w, kv_b, proj_w, proj_b,
           attn_biases, bias_idxs):
    from concourse.bass_utils import run_bass_kernel_spmd

    global LAST_RESULTS
    ll = np.asarray(ll)
    high_attn = np.asarray(high_attn)

    if "nc" not in _CACHE:
        _CACHE["nc"] = _build_nc()
    nc = _CACHE["nc"]

    consts = _prep_consts(
        np.asarray(q_w), np.asarray(q_b), np.asarray(kv_w), np.asarray(kv_b),
        np.asarray(proj_w), np.asarray(proj_b), np.asarray(attn_biases),
        np.asarray(bias_idxs),
    )

    ll16 = ll.reshape(B, C, N).astype(np.float16)
    ha16 = high_attn.reshape(B, C, N).astype(np.float16)

    in_maps = []
    for i in range(NCORES):
        m = {"ll": ll16[BPC * i:BPC * (i + 1)], "ha": ha16[BPC * i:BPC * (i + 1)]}
        m.update(consts)
        in_maps.append(m)

    res = run_bass_kernel_spmd(nc, in_maps, core_ids=list(range(NCORES)),
                               trace=TRACE, tmpdir=TRACE_DIR)
    LAST_RESULTS = {"exec_time_ns": res.exec_time_ns,
                    "scope_times": res.per_core_scope_times}

    out = np.empty((B, C, N), np.float32)
    for i in range(NCORES):
        out[BPC * i:BPC * (i + 1)] = res.results[i]["out"]
    return out.reshape(B, C, RES, RES)
